# revision 12
# baseline (speedup 1.0000x reference)
"""Causal attention (anti-causal masked, faithful to reference) on 8 TRN2 cores.

Sharding: data-parallel over batch (2) x tensor-parallel over heads (16 -> 4
groups of 4 heads). Core c handles batch c//4, heads [ (c%4)*4, (c%4)*4+4 ).

Per-core kernel plan (all shapes hardcoded for B=2, S=2048, D=1024, H=16):
  - host pre-transposes x[b] -> xT [D, S] and weight shards -> wT [D, 256],
    casts matmul operands to fp16 (scores/outputs accumulate in fp32 PSUM).
  - projections: QT/KT computed transposed [c, s] (lhsT=wT, rhs=xT);
    V computed natural [s, c] (lhsT=xT, rhs=wT); biases folded in via a
    K=1 ones-row matmul into the same PSUM accumulation group.
  - scores computed TRANSPOSED: S_T[k, q] = K^T-tile^T Q^T  (both operands
    already in [dh, S] layout; no transposes needed anywhere in the hot loop).
  - exp via ACT with fused scale 1/4 and bias -EXP_SHIFT (fp16 overflow
    guard; cancels exactly in the softmax division). Masked entries are
    zeroed AFTER exp by a multiplicative strict-lower-triangle mask
    (reference keeps only k > q scores; all-masked blocks are skipped).
  - P^T V accumulated per q-chunk with V augmented by a ones column, so the
    softmax denominator falls out of the same matmuls (row 64 of PV psum).
  - final: PE-transpose of out^T tiles -> natural layout, multiply by 1/d
    (per-partition scalar), row 2047 overwritten with mean(V) (reference
    softmaxes an all-(-1e9) row there -> uniform weights).
"""

import numpy as np

import concourse.bass as bass
import concourse.tile as tile
from concourse import bacc, mybir
from concourse.bass_utils import run_bass_kernel_spmd
from concourse.masks import make_identity

F32 = mybir.dt.float32
F16 = mybir.dt.float16
AF = mybir.ActivationFunctionType

B, S, D, H, DH = 2, 2048, 1024, 16, 64
N_CORES = 8
HPC = 4            # heads per core
C = HPC * DH       # channels per core (256)
KC = D // 128      # contraction chunks (8)
EXP_SHIFT = 4.0    # exp(s/4 - 4): keeps fp16 P in range; cancels in division

_CACHE = {}


def _emit(tc, xT, wqT, wkT, wvT, bq, bk, bv, out):
    nc = tc.nc
    DT = F16

    const_p = tc.alloc_tile_pool(name="const", bufs=1)
    xt_p = tc.alloc_tile_pool(name="xt", bufs=KC)
    w_p = tc.alloc_tile_pool(name="w", bufs=3 * KC)
    qk_p = tc.alloc_tile_pool(name="qk", bufs=4)
    v_p = tc.alloc_tile_pool(name="v", bufs=16)
    ex_p = tc.alloc_tile_pool(name="ex", bufs=3)
    ot_p = tc.alloc_tile_pool(name="ot", bufs=6)
    osb_p = tc.alloc_tile_pool(name="osb", bufs=3)
    rs_p = tc.alloc_tile_pool(name="rs", bufs=2)
    ps_mm = tc.alloc_tile_pool(name="psmm", bufs=2, space="PSUM")
    ps_st = tc.alloc_tile_pool(name="psst", bufs=2, space="PSUM")
    ps_pv = tc.alloc_tile_pool(name="pspv", bufs=2, space="PSUM")

    # ---- constants ----
    ident = const_p.tile([128, 128], F32, tag="ident")
    make_identity(nc, ident[:])
    onesrow = const_p.tile([1, 512], DT, tag="onesrow")
    nc.vector.memset(onesrow[:], 1.0)
    onescol = const_p.tile([128, 1], DT, tag="onescol")
    nc.vector.memset(onescol[:], 1.0)
    expb = const_p.tile([128, 1], F32, tag="expb")
    nc.vector.memset(expb[:], -EXP_SHIFT)

    bq_t = const_p.tile([1, C], DT, tag="bq")
    nc.sync.dma_start(bq_t[:], bq[:])
    bk_t = const_p.tile([1, C], DT, tag="bk")
    nc.sync.dma_start(bk_t[:], bk[:])
    bv_t = const_p.tile([1, C], DT, tag="bv")
    nc.sync.dma_start(bv_t[:], bv[:])

    # masks: [128, 2, 512], element (p, m, f) = 1 iff f < p + 128*(m + moff)
    # (strict "k > q" keep-mask for the two diagonal k-tile pairs of a q-chunk)
    masks = []
    for moff in (0, 2):
        mk = const_p.tile([128, 2, 512], DT, tag=f"mask{moff}")
        nc.vector.memset(mk[:], 1.0)
        nc.gpsimd.affine_select(
            out=mk[:],
            in_=mk[:],
            compare_op=mybir.AluOpType.is_ge,
            fill=0.0,
            base=128 * moff - 1,
            pattern=[[128, 2], [-1, 512]],
            channel_multiplier=1,
        )
        masks.append(mk)

    # ---- load x^T and weight shards ----
    xt = []
    for kc in range(KC):
        t = xt_p.tile([128, S], DT, tag="xt")
        nc.sync.dma_start(t[:], xT[kc * 128:(kc + 1) * 128, :])
        xt.append(t)
    wq, wk, wv = [], [], []
    for dst, src, tg in ((wq, wqT, "wq"), (wk, wkT, "wk"), (wv, wvT, "wv")):
        for kc in range(KC):
            t = w_p.tile([128, C], DT, tag=tg)
            nc.sync.dma_start(t[:], src[kc * 128:(kc + 1) * 128, :])
            dst.append(t)

    # ---- projections ----
    # QT/KT in transposed layout [c, s]: tile ct holds channels
    # [128ct, 128ct+128) = heads 2ct (partitions 0-63) and 2ct+1 (64-127).
    QT = [qk_p.tile([128, S], DT, tag="qkt", name=f"QT{i}") for i in range(2)]
    KT = [qk_p.tile([128, S], DT, tag="qkt", name=f"KT{i}") for i in range(2)]
    for dst, w, brow in ((QT, wq, bq_t), (KT, wk, bk_t)):
        for ct in range(2):
            c_sl = slice(ct * 128, (ct + 1) * 128)
            for sc in range(4):
                s_sl = slice(sc * 512, (sc + 1) * 512)
                ps = ps_mm.tile([128, 512], F32, tag="mm")
                for kc in range(KC):
                    nc.tensor.matmul(ps[:], w[kc][:, c_sl], xt[kc][:, s_sl],
                                     start=(kc == 0), stop=False)
                nc.tensor.matmul(ps[:], brow[0:1, c_sl], onesrow[0:1, :],
                                 start=False, stop=True)
                nc.vector.tensor_copy(dst[ct][:, s_sl], ps[:])

    # V natural layout [s, c], augmented: per head 64 V-channels + ones col.
    Vg = []
    for si in range(16):
        s_sl = slice(si * 128, (si + 1) * 128)
        vt = v_p.tile([128, HPC * (DH + 1)], DT, tag="vg")
        vt3 = vt.rearrange("p (h c) -> p h c", h=HPC)
        nc.vector.memset(vt3[:, :, DH:DH + 1], 1.0)
        ps = ps_mm.tile([128, C], F32, tag="mm")
        for kc in range(KC):
            nc.tensor.matmul(ps[:], xt[kc][:, s_sl], wv[kc],
                             start=(kc == 0), stop=False)
        nc.tensor.matmul(ps[:], onesrow[0:1, 0:128], bv_t[0:1, :],
                         start=False, stop=True)
        nc.vector.tensor_copy(vt3[:, :, 0:DH],
                              ps.rearrange("p (h c) -> p h c", h=HPC))
        Vg.append(vt)

    # column-sum of V (for the all-masked last query row): [1, 260]
    psv = ps_mm.tile([1, HPC * (DH + 1)], F32, tag="mm")
    for si in range(16):
        nc.tensor.matmul(psv[:], onescol[:], Vg[si][:],
                         start=(si == 0), stop=(si == 15))
    vmean = const_p.tile([1, HPC * (DH + 1)], F32, tag="vmean")
    nc.scalar.mul(vmean[:], psv[:], 1.0 / S)

    # ---- attention, q-chunk major ----
    for qc in range(4):
        q_sl = slice(qc * 512, (qc + 1) * 512)
        trs = []
        for h in range(HPC):
            ct, po = h // 2, (h % 2) * 64
            pv = ps_pv.tile([DH + 1, 512], F32, tag="pv")
            ks = list(range(4 * qc, 16))
            pairs = [(ks[i], ks[i + 1]) for i in range(0, len(ks), 2)]
            for pi, (ja, jb) in enumerate(pairs):
                st = ps_st.tile([128, 1024], F32, tag="st")
                nc.tensor.matmul(st[:, 0:512],
                                 KT[ct][po:po + 64, ja * 128:(ja + 1) * 128],
                                 QT[ct][po:po + 64, q_sl],
                                 start=True, stop=True)
                nc.tensor.matmul(st[:, 512:1024],
                                 KT[ct][po:po + 64, jb * 128:(jb + 1) * 128],
                                 QT[ct][po:po + 64, q_sl],
                                 start=True, stop=True)
                ex = ex_p.tile([128, 1024], DT, tag="ex")
                nc.scalar.activation(out=ex[:], in_=st[:], func=AF.Exp,
                                     scale=0.25, bias=expb[:])
                if pi < 2:
                    mk = masks[pi]
                    nc.vector.tensor_mul(
                        ex.rearrange("p (m f) -> p m f", m=2),
                        ex.rearrange("p (m f) -> p m f", m=2), mk[:])
                nc.tensor.matmul(pv[:], Vg[ja][:, h * (DH + 1):(h + 1) * (DH + 1)],
                                 ex[:, 0:512], start=(pi == 0), stop=False)
                nc.tensor.matmul(pv[:], Vg[jb][:, h * (DH + 1):(h + 1) * (DH + 1)],
                                 ex[:, 512:1024], start=False,
                                 stop=(pi == len(pairs) - 1))
            ot = ot_p.tile([DH + 1, 512], F32, tag="ot")
            nc.vector.tensor_copy(ot[:], pv[:])
            trs.append(ot)
        # assembly for the 4 q-tiles of this q-chunk: transpose [65, 128]
        # out^T+denominator slices to [128, 65], then scale by 1/denom.
        for t in range(4):
            qt = 4 * qc + t
            osb = osb_p.tile([128, C], F32, tag="osb")
            for h in range(HPC):
                tr = ps_mm.tile([128, DH + 1], F32, tag="mm")
                nc.tensor.transpose(tr[:], trs[h][:, t * 128:(t + 1) * 128],
                                    ident[0:DH + 1, 0:DH + 1])
                rcol = rs_p.tile([128, 1], F32, tag="rs")
                nc.vector.reciprocal(rcol[:], tr[:, DH:DH + 1])
                nc.vector.tensor_scalar_mul(osb[:, h * DH:(h + 1) * DH],
                                            tr[:, 0:DH], rcol[:])
            if qt == 15:
                # overwrite the final query row with mean(V) per head
                # (DMA: DVE ops cannot address a single partition at 127)
                vm3 = vmean.rearrange("o (h c) -> o h c", h=HPC)
                nc.sync.dma_start(
                    osb[127:128, :].rearrange("o (h c) -> o h c", h=HPC),
                    vm3[:, :, 0:DH])
            nc.sync.dma_start(out[qt * 128:(qt + 1) * 128, :], osb[:])

    for p in reversed((const_p, xt_p, w_p, qk_p, v_p, ex_p, ot_p, osb_p,
                       rs_p, ps_mm, ps_st, ps_pv)):
        p.release()


def _build():
    if "nc" in _CACHE:
        return _CACHE["nc"]
    nc = bacc.Bacc("TRN2", target_bir_lowering=False, debug=False,
                   num_devices=N_CORES)
    xT = nc.dram_tensor("xT", [D, S], F16, kind="ExternalInput").ap()
    wqT = nc.dram_tensor("wqT", [D, C], F16, kind="ExternalInput").ap()
    wkT = nc.dram_tensor("wkT", [D, C], F16, kind="ExternalInput").ap()
    wvT = nc.dram_tensor("wvT", [D, C], F16, kind="ExternalInput").ap()
    bq = nc.dram_tensor("bq", [1, C], F16, kind="ExternalInput").ap()
    bk = nc.dram_tensor("bk", [1, C], F16, kind="ExternalInput").ap()
    bv = nc.dram_tensor("bv", [1, C], F16, kind="ExternalInput").ap()
    out = nc.dram_tensor("out", [S, C], F32, kind="ExternalOutput").ap()
    with tile.TileContext(nc) as tc:
        _emit(tc, xT, wqT, wkT, wvT, bq, bk, bv, out)
    nc.compile()
    _CACHE["nc"] = nc
    return nc


def make_in_maps(x, Wq, bq, Wk, bk, Wv, bv):
    in_maps = []
    for c in range(N_CORES):
        b, g = c // HPC, c % HPC
        cols = slice(g * C, (g + 1) * C)
        in_maps.append({
            "xT": np.ascontiguousarray(x[b].T).astype(np.float16),
            "wqT": np.ascontiguousarray(Wq[cols, :].T).astype(np.float16),
            "wkT": np.ascontiguousarray(Wk[cols, :].T).astype(np.float16),
            "wvT": np.ascontiguousarray(Wv[cols, :].T).astype(np.float16),
            "bq": bq[cols].reshape(1, C).astype(np.float16),
            "bk": bk[cols].reshape(1, C).astype(np.float16),
            "bv": bv[cols].reshape(1, C).astype(np.float16),
        })
    return in_maps


def assemble(results):
    out = np.empty((B, S, D), np.float32)
    for c in range(N_CORES):
        b, g = c // HPC, c % HPC
        out[b, :, g * C:(g + 1) * C] = results[c]["out"]
    return out


def kernel(x, Wq, bq, Wk, bk, Wv, bv):
    nc = _build()
    in_maps = make_in_maps(x, Wq, bq, Wk, bk, Wv, bv)
    res = run_bass_kernel_spmd(nc, in_maps, core_ids=list(range(N_CORES)))
    return assemble(res.results)
